# revision 56
# baseline (speedup 1.0000x reference)
"""Batched GAT (GATv2-style attention) Trainium2 Bass kernel, v3.

Sharding: data-parallel over batch (4 graphs x 2 cores); within a graph,
dst nodes split in two halves, one per core (SPMD, one BIR for all cores).

Per core:
  Prologue: input DMAs fanned out across the SP/ACT/Pool issue queues
    (SP serialization was ~1.2us per dma_start). xT loads in quarters so
    phase-A matmuls overlap the DMA; xl_scr store batches alternate
    SP/Pool queues.
  Phase A: xl = x@W_l for all 8192 graph nodes on the TensorEngine in
    bf16, stored as a p-major-row bf16 DRAM table (node n -> row
    (n%128)*64 + n//128 so each store batch is contiguous per
    partition); row N is a poison row (-1000*sign(att)) that padded edge
    slots gather, driving their logits to -inf so exp -> 0 with no mask
    pass. xr rows are computed straight into SBUF in tile-perm order
    from a host-permuted copy of x^T.
  Tile loop (32 tiles x 128 dst rows, degree-sorted, per-tile max degree
  G_t shared across cores so all 8 cores run one BIR), software-
  pipelined G|A|X|B|M|R with explicit stage skew and 3-deep gather
  prefetch (gpool bufs=4):
    - dma_gather of xl[src] rows -> xlg [P, g, 128] bf16 (<=1024
      idx/call: larger calls crash the SWDGE ucode)
    - z = xlg + xr_bcast on the PE into PSUM (identity matmul + a
      broadcast-rhs matmul, accumulated)
    - S1 = cumsum(lrelu(z) * att_bcast)  (custom fused DVE scan op, f32;
      custom DVE ops get no 2x/4x perf modes, so this 1x pass is the
      kernel's critical-path floor at ~84us)
    - e[j,h] = S1[32h+32j*4+31] diffs at head boundaries (+ pad mask)
    - exs = exp(e) [P,g,H] small (ACT); exr = exp(e) broadcast over C
      for the DVE's channels only (ACT only materializes [WSPL:128))
    - dn = sum_j exs  (DVE reduce), rd = 1/dn
    - w = xlg * exp  in-place, split: channels [0,WSPL) multiply on the
      Pool engine reading exs through a stride-0 broadcast view (Pool
      has no 2x modes, so the broadcast is free there); [WSPL,128) on
      the DVE at bf16 2x against the packed exr (engine load balancing:
      DVE/Pool/PE all land ~100-125us busy)
    - psum += w[:, j, :]  via identity matmuls (PE bf16, 1-slot psum)
    - y_all[:, t, :] = psum * rd_bcast with fused row-sum accum (DVE
      scalar_tensor_tensor accum_out); sum(y^2) via one ACT Square+accum
  Epilogue (emitted after the loop; early groups' deps resolve mid-loop
  so engines absorb them in loop slack): per 8-tile group, mu/var on
  Pool, rstd = exp(-0.5*ln(var+eps)) on ACT, normalize on the DVE,
  dma_scatter_add rows to out in <=1024-index calls. (GPSIMD cannot
  touch PSUM, so only SBUF-resident work can move off the DVE; the
  exact DVE/Pool split here is measured, not derived.)

The fused scan op computes running cumsum(lrelu(z)*att) along each
partition row; per-(slot, head) logits are differences of the running sum
at 32-channel boundaries (f32 accumulator => ~1e-6 cancellation error).
"""

import numpy as np
import ml_dtypes

P = 128           # partitions / dst nodes per tile
D = 128           # feature dim (IN_DIM == OUT_DIM == HEADS*C)
H = 4             # heads
C = 32            # channels per head
B = 4             # graphs
N = 8192          # nodes per graph
NCORES = 8
HALVES = NCORES // B          # cores per graph
NSHARD = N // HALVES          # dst nodes per core
NTILES = NSHARD // P

NEG = -1.0e4      # additive mask for padded edge slots
BF16 = ml_dtypes.bfloat16
WSPL = 96         # channels of the weight-multiply offloaded to Pool


# ------------------------------------------------------------ custom DVE op

def _register_scan_op():
    """Fused out = cumsum(max(in0, s0*in0) * in1) along the free axis."""
    from concourse.dve_ops import (DveOp, OPS, CUSTOM_DVE_SPECS,
                                   _SUB_OPCODE_FOR_NAME)
    from concourse.dve_spec import Spec, Src0, Src1, scan, AluOp, maxx, \
        lower, C0, C1
    from concourse.dve_uop import DveOpSpec

    name = "GAT_LRELU_MUL_SCAN"
    if name in _SUB_OPCODE_FOR_NAME:
        return next(o for o in OPS if o.name == name)

    def ref(in0, in1, s0, s1, imm2):
        p = in0.shape[0]
        z = in0.astype(np.float32).reshape(p, -1)
        h = np.maximum(z, s1 * z) * in1.astype(np.float32).reshape(p, -1)
        acc = np.cumsum(h, axis=-1, dtype=np.float32)
        return (acc + np.asarray(s0, np.float32).reshape(-1, 1)
                ).reshape(in0.shape)

    body = scan(AluOp.ADD, maxx(Src0, Src0 * C1) * Src1, init=C0)
    spec = Spec(body=body, reference=ref)
    shas = {}
    for ver in ("v3", "v4"):
        uops = lower(spec, ver=ver)
        s = DveOpSpec(name=name, opcode=31, uops=uops, rd1_en=True)
        shas[ver] = s.sha(ver)
    op = DveOp(name, spec, subdim=False, uops_sha=shas)
    OPS.append(op)
    CUSTOM_DVE_SPECS[name] = spec
    _SUB_OPCODE_FOR_NAME[name] = max(_SUB_OPCODE_FOR_NAME.values()) + 1
    return op


# ----------------------------------------------------------------- host prep

def _shard_edges(src, dst, lo, hi):
    """Edges with dst in [lo,hi) plus self loops; CSR by local dst."""
    sel = (dst >= lo) & (dst < hi)
    s = np.concatenate([src[sel], np.arange(lo, hi, dtype=np.int64)])
    d = np.concatenate([dst[sel], np.arange(lo, hi, dtype=np.int64)]) - lo
    order = np.argsort(d, kind="stable")
    s = s[order]
    deg = np.bincount(d, minlength=hi - lo)
    starts = np.zeros(hi - lo + 1, np.int64)
    np.cumsum(deg, out=starts[1:])
    perm = np.argsort(-deg, kind="stable")
    return s, starts, deg, perm


def interleave16(block):
    """[P, G] per-slot table -> int16 idx array [128, P*G/16] in the
    dma_gather wrapped layout: flat position i = j*P + p -> [i%16, i//16],
    with the 16-partition block replicated 8x (one copy per Q7 core), so
    gathered row i lands at dst[p=i%128, j=i//128]."""
    flat = block.T.reshape(-1)                # i = j*P + p
    m16 = flat.reshape(-1, 16).T.astype(np.int16)
    return np.ascontiguousarray(np.tile(m16, (8, 1)))


def _host_prep(edge_index):
    shards = []
    for b in range(B):
        src = np.asarray(edge_index[b, 0], np.int64)
        dst = np.asarray(edge_index[b, 1], np.int64)
        for hh in range(HALVES):
            lo, hi = hh * NSHARD, (hh + 1) * NSHARD
            s, starts, deg, perm = _shard_edges(src, dst, lo, hi)
            shards.append(dict(s=s, starts=starts, deg=deg, perm=perm,
                               lo=lo, b=b))
    G = np.zeros(NTILES, np.int64)
    for sh in shards:
        ds = sh["deg"][sh["perm"]]
        for t in range(NTILES):
            G[t] = max(G[t], int(ds[t * P:(t + 1) * P].max()))
    G = np.maximum(G, 1)
    sumG = int(G.sum())
    offs = np.zeros(NTILES, np.int64)
    np.cumsum(G[:-1], out=offs[1:])

    for sh in shards:
        s, starts, deg, perm = sh["s"], sh["starts"], sh["deg"], sh["perm"]
        pad_src = np.full((P, sumG), N, np.int64)   # poison row
        perm_g = np.zeros((P, NTILES), np.int64)   # global ids (xr gather)
        perm_l = np.zeros((P, NTILES), np.int64)   # local rows (out scatter)
        for t in range(NTILES):
            off = int(offs[t])
            for p in range(P):
                j = int(perm[t * P + p])
                dj = int(deg[j])
                g0 = int(starts[j])
                pad_src[p, off:off + dj] = s[g0:g0 + dj]
                perm_g[p, t] = j + sh["lo"]
                perm_l[p, t] = j
        # xl table rows are p-major: node n -> row (n%128)*64 + n//128
        prows = np.where(pad_src < N,
                         (pad_src % P) * (N // P) + pad_src // P, N)
        gidx = np.concatenate(
            [interleave16(prows[:, int(offs[t]):int(offs[t]) + int(G[t])])
             for t in range(NTILES)], axis=1)      # [128, 8*sumG]
        sh["gidx"] = gidx
        sh["perm_g"] = perm_g                      # [P, NTILES] global ids
        sh["sc_idx"] = interleave16(perm_l)        # [128, 8*NTILES]
    return shards, G, offs, sumG


# ------------------------------------------------------------- device kernel

def _build_nc(G, offs, sumG, use_bias_lr, use_bias_out, use_gamma, use_beta,
              nn=N, nshard=NSHARD):
    import concourse.bass as bass
    import concourse.tile as tile
    from concourse import bacc, mybir

    scan_op = _register_scan_op()

    f32 = mybir.dt.float32
    bf16 = mybir.dt.bfloat16
    i16 = mybir.dt.int16
    ntiles = len(G)
    nchunks = nn // P
    CB = 8                        # phase A chunks per DRAM store batch

    nc = bacc.Bacc("TRN2", target_bir_lowering=False, debug=False)

    # ---------------- I/O
    xTb = nc.dram_tensor("xTb", [P, nn], bf16, kind="ExternalInput")
    xTp = nc.dram_tensor("xTp", [P, nshard], bf16, kind="ExternalInput")
    Wcat = nc.dram_tensor("Wcat", [P, 2 * D], bf16, kind="ExternalInput")
    bcat = nc.dram_tensor("bcat", [1, 2 * D], bf16, kind="ExternalInput")
    ones1 = nc.dram_tensor("ones1", [1, P], bf16, kind="ExternalInput")
    ident = nc.dram_tensor("ident", [P, P], bf16, kind="ExternalInput")
    att_rep = nc.dram_tensor("att_rep", [P, D], bf16, kind="ExternalInput")
    gamma_rep = nc.dram_tensor("gamma_rep", [P, D], f32, kind="ExternalInput")
    beta_rep = nc.dram_tensor("beta_rep", [P, D], f32, kind="ExternalInput")
    biasv_rep = nc.dram_tensor("biasv_rep", [P, D], f32, kind="ExternalInput")
    gidx = nc.dram_tensor("gidx", [P, 8 * sumG], i16, kind="ExternalInput")
    sc_idx = nc.dram_tensor("sc_idx", [P, 8 * ntiles], i16,
                            kind="ExternalInput")
    padrow = nc.dram_tensor("padrow", [1, D], bf16, kind="ExternalInput")
    out = nc.dram_tensor("out", [nshard, D], f32, kind="ExternalOutput")

    xl_scr = nc.dram_tensor("xl_scr", [nn + 1, D], bf16, kind="Internal")

    add = mybir.AluOpType.add
    mult = mybir.AluOpType.mult
    subtract = mybir.AluOpType.subtract
    divide = mybir.AluOpType.divide
    AF = mybir.ActivationFunctionType
    AX = mybir.AxisListType

    with tile.TileContext(nc) as tc:
        import contextlib
        with contextlib.ExitStack() as ctx:
            consts = ctx.enter_context(tc.tile_pool(name="consts", bufs=1))
            gpool = ctx.enter_context(tc.tile_pool(name="gpool", bufs=3))
            spool = ctx.enter_context(tc.tile_pool(name="spool", bufs=2))
            epool = ctx.enter_context(tc.tile_pool(name="epool", bufs=3))
            smalls = ctx.enter_context(tc.tile_pool(name="smalls", bufs=4))

            # ---- load constants; issue DMAs from several engine queues so
            # the prologue isn't serialized on the SP sequencer.
            def cload(dram, shape, dt, tag, eng=None):
                t = consts.tile(shape, dt, tag=tag)
                (eng or nc.sync).dma_start(t[:], dram[:])
                return t

            # xT loaded in quarters so phase-A matmuls overlap the DMA
            xT_sb = consts.tile([P, nn], bf16, tag="xT")
            QL = nn // 4
            nc.sync.dma_start(xT_sb[:, :QL], xTb[:, :QL])
            W_sb = cload(Wcat, [P, 2 * D], bf16, "W", nc.scalar)
            b_sb = cload(bcat, [1, 2 * D], bf16, "b", nc.scalar) \
                if use_bias_lr else None
            ones_sb = cload(ones1, [1, P], bf16, "ones", nc.scalar) \
                if use_bias_lr else None
            id_sb = cload(ident, [P, P], bf16, "id", nc.scalar)
            att_sb = cload(att_rep, [P, D], bf16, "att", nc.scalar)
            gam_sb = cload(gamma_rep, [P, D], f32, "gam", nc.scalar) \
                if use_gamma else None
            bet_sb = cload(beta_rep, [P, D], f32, "bet", nc.scalar) \
                if use_beta else None
            bv_sb = cload(biasv_rep, [P, D], f32, "bv", nc.scalar) \
                if use_bias_out else None
            # poison row for padded slots: row N of xl_scr
            pr_sb = consts.tile([1, D], bf16, tag="pr")
            nc.scalar.dma_start(pr_sb[:], padrow[:])
            nc.scalar.dma_start(xl_scr[nn:nn + 1, :], pr_sb[:])
            # index tables + permuted xr source, issued on otherwise-idle
            # queues up front
            gi_sb = consts.tile([P, 8 * sumG], i16, tag="gi")
            half = 4 * sumG
            nc.gpsimd.dma_start(gi_sb[:, :half], gidx[:, :half])
            nc.gpsimd.dma_start(gi_sb[:, half:], gidx[:, half:])
            sci_sb = cload(sc_idx, [P, 8 * ntiles], i16, "sci", nc.scalar)
            xTp_sb = cload(xTp, [P, nshard], bf16, "xTp", nc.gpsimd)

            eps_sb = consts.tile([P, 1], f32, tag="eps")
            nc.vector.memset(eps_sb[:], 1e-5)
            sy_all = consts.tile([P, ntiles], f32, tag="sy_all")
            sy2_all = consts.tile([P, ntiles], f32, tag="sy2_all")
            sq_scrap = consts.tile([P, D], f32, tag="sq_scrap")
            Gmax = int(max(G))
            S1a = consts.tile([P, Gmax * D + 64], f32, tag="S1a")
            S1b = consts.tile([P, Gmax * D + 64], f32, tag="S1b")
            nc.vector.memset(S1a[:, 0:1], 0.0)
            nc.vector.memset(S1b[:, 0:1], 0.0)
            xr_all = consts.tile([P, ntiles, D], bf16, tag="xr_all")
            y_all = consts.tile([P, ntiles, D], f32, tag="y_all")

            # ---- Phase A: xl bf16 row table (p-major rows) + xr_all
            # directly from host-permuted xTp. Copies on ACT, 2-chunk psum.
            phA = contextlib.ExitStack()
            psA = phA.enter_context(tc.tile_pool(name="psA", bufs=4,
                                                 space="PSUM"))
            stA = phA.enter_context(tc.tile_pool(name="stA", bufs=4))
            KB = nn // P  # 64 table rows per partition
            CPQ = QL // P  # phase-A chunks per xT quarter
            for c0 in range(0, nchunks, CB):
                if c0 % CPQ == 0 and c0 + CPQ < nchunks:
                    q0 = (c0 // CPQ + 1) * QL
                    nc.sync.dma_start(xT_sb[:, q0:q0 + QL], xTb[:, q0:q0 + QL])
                st = stA.tile([P, CB, D], bf16, tag="stg")
                for k in range(0, CB, 4):
                    ps = psA.tile([P, 4, D], f32)
                    for kk in range(4):
                        cix = c0 + k + kk
                        nc.tensor.matmul(ps[:, kk, :],
                                         lhsT=xT_sb[:, cix * P:(cix + 1) * P],
                                         rhs=W_sb[:, :D], start=True,
                                         stop=not use_bias_lr)
                        if use_bias_lr:
                            nc.tensor.matmul(ps[:, kk, :],
                                             lhsT=ones_sb[:1, :],
                                             rhs=b_sb[:1, :D], start=False,
                                             stop=True)
                    if (k // 4) % 2 == 0:
                        nc.scalar.copy(st[:, k:k + 4, :], ps[:])
                    else:
                        nc.vector.tensor_copy(st[:, k:k + 4, :], ps[:])
                st_eng = nc.sync if (c0 // CB) % 2 == 0 else nc.gpsimd
                st_eng.dma_start(
                    xl_scr[:nn].rearrange("(p k) d -> p k d", p=P)
                    [:, c0:c0 + CB, :], st[:])
            # xr rows per tile, perm order, straight into SBUF
            for t0 in range(0, ntiles, 4):
                ps = psA.tile([P, 4, D], f32)
                for kk in range(4):
                    t = t0 + kk
                    nc.tensor.matmul(ps[:, kk, :],
                                     lhsT=xTp_sb[:, t * P:(t + 1) * P],
                                     rhs=W_sb[:, D:], start=True,
                                     stop=not use_bias_lr)
                    if use_bias_lr:
                        nc.tensor.matmul(ps[:, kk, :], lhsT=ones_sb[:1, :],
                                         rhs=b_sb[:1, D:], start=False,
                                         stop=True)
                if (t0 // 4) % 2 == 0:
                    nc.scalar.copy(xr_all[:, t0:t0 + 4, :], ps[:])
                else:
                    nc.vector.tensor_copy(xr_all[:, t0:t0 + 4, :], ps[:])
            phA.close()
            psZ = ctx.enter_context(
                tc.tile_pool(name="psZ", bufs=2, space="PSUM"))
            psB = ctx.enter_context(
                tc.tile_pool(name="psB", bufs=2, space="PSUM"))

            GCH = 8
            GGH = 8    # j-slots per dma_gather call (1024 idxs)

            # ---- Tile loop, software-pipelined across tiles:
            #   G(t) gather | A(t) z+scan+e-diff (DVE) | X(t) exp (ACT) |
            #   B(t) dn+rcp+wmult (DVE) | M(t) psum matmuls (PE) |
            #   R(t) rd-scale+bn (DVE)
            # Emission order skews stages so each engine stays busy.
            xlg_t = [None] * ntiles
            exr_t = [None] * ntiles
            exs_t = [None] * ntiles
            e_t = [None] * ntiles
            rd_t = [None] * ntiles
            pt_t = [None] * ntiles

            def stage_G(t):
                # gathers run per tile PAIR (even t): the index table is
                # contiguous across tiles, so one call sequence covers both
                # and the per-call SWDGE fixed cost amortizes better
                g = int(G[t]) + (int(G[t + 1]) if t + 1 < ntiles else 0)
                off = int(offs[t])
                xlg = gpool.tile([P, g, D], bf16, tag="xlg")
                xlg_t[t] = xlg[:, :int(G[t]), :]
                if t + 1 < ntiles:
                    xlg_t[t + 1] = xlg[:, int(G[t]):, :]
                for j0 in range(0, g, GGH):
                    cn = min(GGH, g - j0)
                    nc.gpsimd.dma_gather(
                        xlg[:, j0:j0 + cn, :], xl_scr[:],
                        gi_sb[:, 8 * (off + j0):8 * (off + j0 + cn)],
                        P * cn, P * cn, D)

            def stage_A(t):
                # z = xlg + xr_bcast on the PE into PSUM (2 matmuls per
                # 512-col half), then seed-chained fused scans per chunk.
                g = int(G[t])
                gd = g * D
                xlg = xlg_t[t]
                xrb = xr_all[:, t, :]
                S1 = S1a if t % 2 == 0 else S1b
                CHJ = 12          # j-slots per psum chunk (1536 cols)
                for j0 in range(0, g, CHJ):
                    m = min(CHJ, g - j0)
                    psz = psZ.tile([P, 3, 512], f32)
                    for half in range(0, m, 4):
                        mh = min(4, m - half)
                        hv = psz[:, half // 4, :mh * D]\
                            .rearrange("p (j d) -> p j d", j=mh)
                        nc.tensor.matmul(
                            hv, lhsT=id_sb[:],
                            rhs=xlg[:, j0 + half:j0 + half + mh, :],
                            start=True, stop=False)
                        nc.tensor.matmul(
                            hv, lhsT=id_sb[:],
                            rhs=xrb.unsqueeze(1).to_broadcast([P, mh, D]),
                            start=False, stop=True)
                    base = j0 * D
                    nc.vector._custom_dve(
                        scan_op,
                        out=S1[:, 1 + base:1 + base + m * D]
                        .rearrange("p (j d) -> p j d", j=m),
                        in0=psz[:, :, :].rearrange("p a b -> p (a b)")
                        [:, :m * D].rearrange("p (j d) -> p j d", j=m),
                        in1=att_sb[:].unsqueeze(1).to_broadcast([P, m, D]),
                        s0=S1[:, base:base + 1], s1=0.2)
                # e[p,j,h] = S1[32 + j*128 + 32h] - S1[j*128 + 32h]
                hi_v = S1[:, 32:32 + gd].rearrange(
                    "p (g h c) -> p g h c", g=g, h=H)[:, :, :, 0:1]
                lo_v = S1[:, 0:gd].rearrange(
                    "p (g h c) -> p g h c", g=g, h=H)[:, :, :, 0:1]
                e = epool.tile([P, g, H], f32, tag="e")
                e_t[t] = e
                e4 = e[:].unsqueeze(3)
                nc.vector.tensor_tensor(out=e4, in0=hi_v, in1=lo_v,
                                        op=subtract)

            HS = WSPL // C        # heads multiplied on Pool (head-aligned)

            def stage_X(t):
                # small exp [P, g, H]: feeds the denominators and the Pool
                # half of the weight-multiply (Pool has no 2x modes, so a
                # stride-0 broadcast view costs it nothing extra)
                g = int(G[t])
                exs = epool.tile([P, g, H], bf16, tag="exs")
                exs_t[t] = exs
                nc.scalar.activation(out=exs[:], in_=e_t[t][:], func=AF.Exp)
                if HS < H:
                    # broadcast exp only for the DVE's channels (2x mode
                    # needs a packed in1)
                    exr = epool.tile([P, g, D - WSPL], bf16, tag="exr")
                    exr_t[t] = exr
                    nc.scalar.activation(
                        out=exr[:].rearrange("p g (h c) -> p g h c",
                                             h=H - HS),
                        in_=e_t[t][:, :, HS:].unsqueeze(3)
                            .to_broadcast([P, g, H - HS, C]),
                        func=AF.Exp)

            def stage_B(t):
                xlg, exs = xlg_t[t], exs_t[t]
                dn = smalls.tile([P, H, 1], f32, tag="dn")
                nc.vector.tensor_reduce(
                    out=dn[:],
                    in_=exs[:].transpose([0, 2, 1]),
                    axis=AX.X, op=add)
                rd = smalls.tile([P, H, 1], f32, tag="rd")
                rd_t[t] = rd
                nc.vector.reciprocal(rd[:], dn[:])
                g = int(G[t])
                if HS:
                    nc.gpsimd.tensor_tensor(
                        out=xlg[:, :, :WSPL]
                            .rearrange("p g (h c) -> p g h c", h=HS),
                        in0=xlg[:, :, :WSPL]
                            .rearrange("p g (h c) -> p g h c", h=HS),
                        in1=exs[:, :, :HS].unsqueeze(3)
                            .to_broadcast([P, g, HS, C]),
                        op=mult)
                if HS < H:
                    nc.vector.tensor_tensor(out=xlg[:, :, WSPL:],
                                            in0=xlg[:, :, WSPL:],
                                            in1=exr_t[t][:], op=mult)

            def stage_M(t):
                g = int(G[t])
                xlg = xlg_t[t]
                pt = psB.tile([P, D], f32)
                pt_t[t] = pt
                for j in range(g):
                    nc.tensor.matmul(pt[:], lhsT=id_sb[:],
                                     rhs=xlg[:, j, :],
                                     start=(j == 0), stop=(j == g - 1))

            def stage_R(t):
                pt, rd = pt_t[t], rd_t[t]
                y = y_all[:, t, :]
                yv = y.rearrange("p (h c) -> p h c", h=H)
                nc.vector.scalar_tensor_tensor(
                    out=yv, in0=pt[:].rearrange("p (h c) -> p h c", h=H),
                    scalar=1.0, in1=rd[:].to_broadcast([P, H, C]),
                    op0=mult, op1=mult,
                    accum_out=sy_all[:, t:t + 1])
                if use_bias_out:
                    nc.vector.tensor_tensor(out=y, in0=y, in1=bv_sb[:],
                                            op=add)
                    nc.scalar.activation(out=sq_scrap[:], in_=y,
                                         func=AF.Copy,
                                         accum_out=sy_all[:, t:t + 1])
                nc.scalar.activation(out=sq_scrap[:], in_=y,
                                     func=AF.Square,
                                     accum_out=sy2_all[:, t:t + 1])
                xlg_t[t] = exr_t[t] = exs_t[t] = e_t[t] = rd_t[t] = \
                    pt_t[t] = None

            # ---- epilogue, interleaved into the pipeline per GCH-tile group:
            # layernorm stats from (sum y, sum y^2), normalize, scatter.
            mu = smalls.tile([P, ntiles], f32, tag="mu")
            var = smalls.tile([P, ntiles], f32, tag="var")
            ey2 = smalls.tile([P, ntiles], f32, tag="ey2")
            sd = smalls.tile([P, ntiles], f32, tag="sd")

            def stage_E(t0):
                cn = min(GCH, ntiles - t0)
                sl = slice(t0, t0 + cn)
                nc.gpsimd.tensor_scalar_mul(mu[:, sl], sy_all[:, sl], 1.0 / D)
                nc.gpsimd.tensor_tensor(out=var[:, sl], in0=mu[:, sl],
                                        in1=mu[:, sl], op=mult)
                nc.gpsimd.tensor_scalar_mul(ey2[:, sl], sy2_all[:, sl],
                                            1.0 / D)
                nc.gpsimd.tensor_tensor(out=var[:, sl], in0=ey2[:, sl],
                                        in1=var[:, sl], op=subtract)
                # rstd = exp(-0.5*ln(var+eps)); Ln/Exp share an ACT table
                nc.scalar.activation(out=sd[:, sl], in_=var[:, sl],
                                     func=AF.Ln, bias=eps_sb[:, :1],
                                     scale=1.0)
                nc.scalar.activation(out=sd[:, sl], in_=sd[:, sl],
                                     func=AF.Exp, scale=-0.5)
                for t in range(t0, t0 + cn):
                    y = y_all[:, t, :]
                    nc.gpsimd.tensor_scalar(
                        out=y, in0=y,
                        scalar1=mu[:, t:t + 1],
                        scalar2=sd[:, t:t + 1],
                        op0=subtract, op1=mult)
                    if use_gamma:
                        nc.gpsimd.tensor_tensor(out=y, in0=y, in1=gam_sb[:],
                                                op=mult)
                    if use_beta:
                        nc.gpsimd.tensor_tensor(out=y, in0=y, in1=bet_sb[:],
                                                op=add)
                # out is zero-initialized; unique indices ->
                # scatter-add == write
                nc.gpsimd.dma_scatter_add(
                    out[:], y_all[:, t0:t0 + cn, :],
                    sci_sb[:, 8 * t0:8 * (t0 + cn)], P * cn, P * cn, D)

            stage_G(0)
            stage_G(2)
            for tt in range(ntiles + 2):
                if tt < ntiles:
                    stage_A(tt)
                    if tt % 2 == 0 and tt + 4 < ntiles:
                        stage_G(tt + 4)
                    stage_X(tt)
                if tt >= 1 and tt - 1 < ntiles:
                    stage_B(tt - 1)
                    stage_M(tt - 1)
                if tt >= 2:
                    stage_R(tt - 2)
            for t0 in range(0, ntiles, GCH):
                stage_E(t0)

    nc.compile()
    return nc


# ------------------------------------------------------------------- runner

def _core_inputs(sh, x, W_l, b_l, W_r, b_r, att, bias, gamma, beta):
    b = sh["b"]
    xTb = np.ascontiguousarray(
        np.asarray(x[b], np.float32).T).astype(BF16)                # [D, N]
    Wcat = np.concatenate([np.asarray(W_l, np.float32),
                           np.asarray(W_r, np.float32)],
                          axis=1).astype(BF16)                      # [D, 2D]
    bcat = np.concatenate([np.asarray(b_l, np.float32),
                           np.asarray(b_r, np.float32)])[None, :]\
        .astype(BF16)                                               # [1, 2D]
    att_rep = np.broadcast_to(
        np.asarray(att, np.float32).reshape(1, D), (P, D)).astype(BF16)
    perm_cols = sh["perm_g"].T.reshape(-1)        # col t*128+p = node id
    xTp = np.ascontiguousarray(
        np.asarray(x[b], np.float32).T[:, perm_cols]).astype(BF16)
    return {
        "xTb": xTb, "xTp": xTp, "Wcat": Wcat, "bcat": bcat,
        "ones1": np.ones((1, P), BF16),
        "ident": np.eye(P).astype(BF16),
        "att_rep": att_rep,
        "gamma_rep": np.broadcast_to(np.asarray(gamma, np.float32),
                                     (P, D)).copy(),
        "beta_rep": np.broadcast_to(np.asarray(beta, np.float32),
                                    (P, D)).copy(),
        "biasv_rep": np.broadcast_to(np.asarray(bias, np.float32),
                                     (P, D)).copy(),
        "gidx": sh["gidx"], "sc_idx": sh["sc_idx"],
        "padrow": np.where(np.asarray(att, np.float32).reshape(1, D) >= 0,
                           -1000.0, 1000.0).astype(BF16),
    }


LAST_EXEC_NS = None
LAST_PROFILE = None


def _build_all(x, edge_index, W_l, b_l, W_r, b_r, att, bias, gamma, beta):
    x = np.asarray(x, np.float32)
    edge_index = np.asarray(edge_index)
    shards, G, offs, sumG = _host_prep(edge_index.astype(np.int64))

    use_bias_lr = not (np.all(np.asarray(b_l) == 0)
                       and np.all(np.asarray(b_r) == 0))
    use_bias_out = not np.all(np.asarray(bias) == 0)
    use_gamma = not np.all(np.asarray(gamma) == 1)
    use_beta = not np.all(np.asarray(beta) == 0)

    nc = _build_nc(G, offs, sumG, use_bias_lr, use_bias_out, use_gamma,
                   use_beta)
    in_maps = [_core_inputs(sh, x, W_l, b_l, W_r, b_r, att, bias,
                            gamma, beta)
               for sh in shards]
    return nc, shards, in_maps


def kernel(x, edge_index, W_l, b_l, W_r, b_r, att, bias, gamma, beta):
    global LAST_EXEC_NS, LAST_PROFILE
    from concourse.bass_utils import run_bass_kernel_spmd

    nc, shards, in_maps = _build_all(x, edge_index, W_l, b_l, W_r, b_r,
                                     att, bias, gamma, beta)
    res = run_bass_kernel_spmd(nc, in_maps, core_ids=list(range(NCORES)))
    LAST_EXEC_NS = res.exec_time_ns
    LAST_PROFILE = res.profile_json
    out = np.zeros((B, N, D), np.float32)
    for ci, sh in enumerate(shards):
        out[sh["b"], sh["lo"]:sh["lo"] + NSHARD] = res.results[ci]["out"]
    return out

